# revision 16
# baseline (speedup 1.0000x reference)
"""PowerSpectrum kernel for Trainium2 (Bass/Tile), SPMD over 8 NeuronCores.

Math (per environment j, per angular channel l):
  c = se_l[j].reshape(A, 2l+1)                 A = 32 combined (species, radial)
  ps = c @ c.T / sqrt(2l+1)                    [A, A] symmetric
  packed_l = triu(ps) * (1 or sqrt2) padded to 544
  out[j] = concat_l packed_l / ||concat_l packed_l||_2

Strategy:
  - Host concatenates the 7 se_l inputs into v = [J, 32, 49] (49 = sum 2l+1)
    so a single broadcast-multiply per triangle row a covers all l at once.
  - Data-parallel over J: J=30000 -> 3750 envs per core.
  - Per 128-env tile (envs on partitions):
      products P[:, pair(a,b), q] = v[:, a, q] * v[:, b, q]  (32 DVE ops, one
      per row a, exact upper triangle, contiguous in packed pair order)
      7 strided segmented reduces (axis=X over each l's m-slice) write the
      packed [128, 528] block of each l directly.
      FAC multiply (replicated constant, includes sqrt2 off-diag and
      1/sqrt(2l+1)), fused square+row-sum on ScalarE for the norm, then
      reciprocal-sqrt with one Newton step and a final per-row scale.
"""

import os
import sys

sys.path.insert(0, "/opt/trn_rl_repo")
sys.path.insert(0, "/opt/pypackages")

import math
import numpy as np

J_TOTAL = 30000
N_CORES = 8
JC = J_TOTAL // N_CORES  # 3750 envs per core
A = 32
NL = 7
Q = 49  # sum of 2l+1 for l=0..6
NPACK = 528
NPAD = 544
OUTW = NL * NPAD  # 3808

# q-offsets of each l's m-slice within the concatenated 49
QOFF = [0, 1, 4, 9, 16, 25, 36]

# rowstart(a) = index of pair (a, a) in row-major triu order
ROWSTART = [a * A - a * (a - 1) // 2 for a in range(A)]

_NC_CACHE = {}


def _build_fac():
    """[128, 3808] replicated row: (1 or sqrt2)/sqrt(2l+1), 0 on pad cols."""
    ti, tj = np.triu_indices(A)
    base = np.where(ti == tj, 1.0, math.sqrt(2.0)).astype(np.float32)
    row = np.zeros((NL, NPAD), dtype=np.float32)
    for l in range(NL):
        row[l, :NPACK] = base / math.sqrt(2 * l + 1)
    row = row.reshape(OUTW)
    return np.ascontiguousarray(np.broadcast_to(row, (128, OUTW)))


def build_nc(jc=JC):
    """Build the single-core Bass program for a jc-environment slice."""
    import concourse.bacc as bacc
    import concourse.tile as tile
    from concourse import mybir

    f32 = mybir.dt.float32
    nc = bacc.Bacc()
    v_d = nc.dram_tensor("v", [jc, A, Q], f32, kind="ExternalInput")
    fac_d = nc.dram_tensor("fac", [128, OUTW], f32, kind="ExternalInput")
    out_d = nc.dram_tensor("out", [jc, OUTW], f32, kind="ExternalOutput")

    P = 128
    ntiles = (jc + P - 1) // P
    # Triangle rows split into chunks, each assigned to one engine: the DVE
    # (1 elem/cycle @0.96GHz) and GpSimd (~2.6 cyc/elem @1.2GHz) split the
    # product work ~60/40 by element count. Each chunk has its own products
    # buffer so the two engines never write the same tile.
    ENGINE_CHUNKS = [(0, 5, "gp"), (5, 11, "gp"), (11, A, "dve")]
    def _npair(lo, hi):
        return (ROWSTART[hi] if hi < A else NPACK) - ROWSTART[lo]

    with tile.TileContext(nc) as tc:
        with (
            tc.tile_pool(name="singles", bufs=1) as singles,
            tc.tile_pool(name="vt", bufs=2) as vpool,
            tc.tile_pool(name="vtps", bufs=2, space="PSUM") as pspool,
            tc.tile_pool(name="prod", bufs=1) as ppool,
            tc.tile_pool(name="packed", bufs=3) as opool,
            tc.tile_pool(name="small", bufs=4) as spool,
        ):
            fac_t = singles.tile([P, OUTW], f32)
            nc.gpsimd.dma_start(out=fac_t[:], in_=fac_d[:])
            sq_t = singles.tile([P, OUTW], mybir.dt.bfloat16)  # square scratch
            prev_dve_last = None

            for it in range(ntiles):
                s = it * P
                e = min(s + P, jc)
                ts = e - s

                vt = vpool.tile([P, A, Q], f32)
                nc.gpsimd.dma_start(out=vt[:ts], in_=v_d[s:e])
                # PSUM copy of v: DVE products read their broadcast operand
                # from PSUM (own read port) so they only use one SBUF read
                # port and do not contend with GpSimd's shared SBUF port.
                vt_ps = pspool.tile([P, A, Q], f32)
                nc.scalar.copy(out=vt_ps[:ts], in_=vt[:ts])

                packed = opool.tile([P, OUTW], f32)
                # zero the 16 pad columns of each l block
                pad_view = packed.rearrange("p (l c) -> p l c", c=NPAD)[
                    :ts, :, NPACK:
                ]
                nc.vector.memset(pad_view, 0.0)

                prods = []
                gp_first = None
                dve_last = None
                for ci, (lo, hi, eng) in enumerate(ENGINE_CHUNKS):
                    npair = _npair(lo, hi)
                    prod = ppool.tile([P, npair, Q], f32, tag=f"prod{ci}")
                    prods.append(prod)
                    base = ROWSTART[lo]
                    engine = nc.gpsimd if eng == "gp" else nc.vector
                    bcast_src = vt if eng == "gp" else vt_ps
                    # products, one op per triangle row
                    for a in range(lo, hi):
                        n = A - a
                        off = ROWSTART[a] - base
                        ins = engine.tensor_mul(
                            out=prod[:ts, off : off + n, :],
                            in0=bcast_src[:ts, a : a + 1, :].broadcast_to((ts, n, Q)),
                            in1=vt[:ts, a:, :],
                        )
                        if eng == "gp" and gp_first is None:
                            gp_first = ins
                        if eng == "dve":
                            dve_last = ins
                # Phase GpSimd against DVE: this tile's GpSimd products may
                # only start once the previous tile's DVE products are done,
                # so GpSimd overlaps the (single-SBUF-port) reduce phase
                # instead of contending with DVE's 2-port product phase.
                if prev_dve_last is not None and gp_first is not None:
                    tile.add_dep_helper(
                        prev_dve_last.ins,
                        gp_first.ins,
                        sync=True,
                        reason="phase gp products into dve reduce window",
                    )
                prev_dve_last = dve_last
                # segmented reduce per (chunk, l), strided over the pair dim
                for ci, (lo, hi, eng) in enumerate(ENGINE_CHUNKS):
                    npair = _npair(lo, hi)
                    base = ROWSTART[lo]
                    for l in range(NL):
                        m = 2 * l + 1
                        q0 = QOFF[l]
                        nc.vector.tensor_reduce(
                            out=packed[:ts, l * NPAD + base : l * NPAD + base + npair],
                            in_=prods[ci][:ts, :npair, q0 : q0 + m],
                            axis=mybir.AxisListType.X,
                            op=mybir.AluOpType.add,
                        )

                # FAC scale (in place)
                nc.vector.tensor_mul(out=packed[:ts], in0=packed[:ts], in1=fac_t[:ts])

                # norm^2 per row via fused square + accumulate on ScalarE
                n2 = spool.tile([P, 1], f32, tag="n2")
                nc.scalar.activation(
                    out=sq_t[:ts],
                    in_=packed[:ts],
                    func=mybir.ActivationFunctionType.Square,
                    accum_out=n2[:ts],
                )
                # r = 1/sqrt(n2), Newton-refined
                r0 = spool.tile([P, 1], f32, tag="r0")
                nc.scalar.sqrt(out=r0[:ts], in_=n2[:ts])
                rr = spool.tile([P, 1], f32, tag="rr")
                nc.vector.reciprocal(out=rr[:ts], in_=r0[:ts])
                t1 = spool.tile([P, 1], f32, tag="t1")
                nc.vector.tensor_mul(out=t1[:ts], in0=rr[:ts], in1=rr[:ts])
                nc.vector.tensor_mul(out=t1[:ts], in0=t1[:ts], in1=n2[:ts])
                nc.vector.tensor_scalar(
                    out=t1[:ts],
                    in0=t1[:ts],
                    scalar1=-0.5,
                    scalar2=1.5,
                    op0=mybir.AluOpType.mult,
                    op1=mybir.AluOpType.add,
                )
                nc.vector.tensor_mul(out=t1[:ts], in0=t1[:ts], in1=rr[:ts])

                # final scale on ScalarE (in place), then store
                nc.scalar.mul(out=packed[:ts], in_=packed[:ts], mul=t1[:ts])
                nc.gpsimd.dma_start(out=out_d[s:e], in_=packed[:ts])

    nc.finalize()
    return nc


def _get_nc(jc=JC):
    if jc not in _NC_CACHE:
        _NC_CACHE[jc] = build_nc(jc)
    return _NC_CACHE[jc]


def _concat_inputs(ses):
    j = ses[0].shape[0]
    return np.concatenate(
        [np.asarray(se, dtype=np.float32).reshape(j, A, 2 * l + 1)
         for l, se in enumerate(ses)],
        axis=2,
    )


def kernel(se0, se1, se2, se3, se4, se5, se6):
    from concourse.bass_utils import run_bass_kernel_spmd

    ses = [se0, se1, se2, se3, se4, se5, se6]
    v = _concat_inputs(ses)
    fac = _build_fac()
    nc = _get_nc(JC)
    in_maps = [
        {"v": np.ascontiguousarray(v[c * JC : (c + 1) * JC]), "fac": fac}
        for c in range(N_CORES)
    ]
    res = run_bass_kernel_spmd(nc, in_maps, list(range(N_CORES)))
    out = np.concatenate([res.results[c]["out"] for c in range(N_CORES)], axis=0)
    return out.astype(np.float32)
